# revision 34
# baseline (speedup 1.0000x reference)
"""Trainium2 Bass kernel for the DAMPS spectral-fusion module.

Takes the full (unsharded) inputs, shards rows across 8 NeuronCores
(pure data parallel), and runs a fused matmul-only reformulation:

  proj + rFFT + phase-rotation  ->  one [raw, 128] matrix per modality
  (spectral packing: p0 = DC (unrotated), p1..63 = Re A_k, p64 = Nyquist
  (unrotated), p65..127 = Im A_k; cos(phi) for DC/Nyquist is folded into
  the output matrix because irfft ignores the imaginary part there)

  msc mask chain               ->  elementwise on [128, rows] tiles
                                   + tiny matmuls for the pair-sum
                                   (|A|^2) and the bin->dim mask expand
  masked irfft                 ->  one [128, 128] fp16 output matrix

Layout: the host pre-transposes each core's row-shard to [raw, rows]
in fp16 (half the HBM bytes of fp32; the 2e-2 tolerance leaves room)
and packs it block-major so each row-block is one fully contiguous
[128, kc*RB] region -> large DMA descriptors.

Schedule: software-pipelined across row blocks with a 3-deep stage
skew (fwd b | pair-sum b-1 | mask-expand b-2 | inverse b-3) so the
tensor engine never waits on the elementwise msc chain — every
cross-engine dependency gets a full iteration of slack.  This keeps
the PE HAM clock-gate warm and the DMA queues streaming.
"""

import math

import numpy as np

N_ITEMS = 30000
D = 128
F = 65
RAW_IMG = 1024
RAW_TXT = 768
N_CORES = 8
ROWS_CORE = N_ITEMS // N_CORES          # 3750
KC_IMG = RAW_IMG // 128                 # 8
KC_TXT = RAW_TXT // 128                 # 6
EPS = 1e-8

# row blocks per core: small first block starts the PE early; uniform after
RBS = [256, 512, 512, 512, 512, 512, 512, 422]
assert sum(RBS) == ROWS_CORE
NB = len(RBS)
RB_MAIN = 512
BLOCKS = []
_r0 = 0
for _rb in RBS:
    BLOCKS.append((_r0, _rb))
    _r0 += _rb

_CACHE = {}


def _bin_of_dim():
    """spectral dim (0..127) -> frequency bin (0..64)"""
    b = np.zeros(128, np.int64)
    b[0] = 0
    b[64] = 64
    b[1:64] = np.arange(1, 64)
    b[65:128] = np.arange(1, 64)
    return b


def _host_consts(W_img, b_img, W_txt, b_txt, avg_R, psi, lambda_weights):
    """Build the fused constant matrices (float64 internally)."""

    n = np.arange(D)
    k = np.arange(F)
    theta = 2.0 * np.pi * np.outer(k, n) / D          # [65, 128]
    phi = (avg_R.astype(np.float64) * 0.5 + psi.astype(np.float64))  # [65]
    s = 1.0 / math.sqrt(D)

    def dmat(sign):
        Dm = np.zeros((128, D))
        Dm[0, :] = s
        Dm[64, :] = s * np.cos(theta[64])
        a = theta[1:64] + sign * phi[1:64, None]
        Dm[1:64, :] = s * np.cos(a)
        Dm[65:128, :] = -s * np.sin(a)
        return Dm

    Dimg = dmat(+1.0)
    Dtxt = dmat(-1.0)

    GimgT = (W_img.astype(np.float64) @ Dimg.T).astype(np.float32)  # [1024,128]
    GtxtT = (W_txt.astype(np.float64) @ Dtxt.T).astype(np.float32)  # [768,128]
    bias_img = (Dimg @ b_img.astype(np.float64)).astype(np.float32)
    bias_txt = (Dtxt @ b_txt.astype(np.float64)).astype(np.float32)

    cphi = np.cos(phi)
    Mout = np.zeros((128, D))
    Mout[0, :] = s * cphi[0]
    Mout[64, :] = s * cphi[64] * np.cos(theta[64])
    Mout[1:64, :] = 2.0 * s * np.cos(theta[1:64])
    Mout[65:128, :] = -2.0 * s * np.sin(theta[1:64])
    Mout = Mout.astype(np.float32)

    lw = lambda_weights.astype(np.float64)
    e = np.exp(lw - lw.max())
    lam = e / e.sum()
    lam0, lam1 = float(lam[0]), float(lam[1])

    bod = _bin_of_dim()
    epair = np.zeros((128, F), np.float32)
    epair[np.arange(F), np.arange(F)] = 1.0            # cos^2 / DC / Nyq
    epair[64 + np.arange(1, 64), np.arange(1, 64)] = 1.0  # sin^2
    # e2 expand matrix maps r[bin] -> -c1*eps*r at each spectral dim; the
    # constant (c0[bin]+c1) term is applied later as a per-partition scalar:
    #   g_exp[d] = (c0[bod d]+c1) - c1*eps*r[bod d]   (== c0 + c1*msc)
    e2 = np.zeros((F, 128), np.float32)
    e2[bod, np.arange(128)] = -lam1 * EPS
    return GimgT, GtxtT, bias_img, bias_txt, Mout, lam0, lam1, epair, e2, bod


def _build_nc():
    """Build (once) the Bass/Tile program for one core's row shard."""
    from contextlib import ExitStack

    import concourse.bass as bass
    import concourse.tile as tile
    from concourse import mybir

    f32 = mybir.dt.float32
    f16 = mybir.dt.float16
    bf16 = mybir.dt.bfloat16
    AF = mybir.ActivationFunctionType

    nc = bass.Bass("TRN2", target_bir_lowering=False, debug=False)

    # block-major fp16 table shards, flat with per-block offsets (graded RB)
    xi = nc.dram_tensor(
        "xi", [128 * RAW_IMG // 128 * ROWS_CORE], f16, kind="ExternalInput"
    ).ap()
    xt = nc.dram_tensor(
        "xt", [128 * RAW_TXT // 128 * ROWS_CORE], f16, kind="ExternalInput"
    ).ap()
    # fused projection matrices, [k, chunk, m] fp16
    g_img = nc.dram_tensor("g_img", [128, KC_IMG, 128], f16, kind="ExternalInput").ap()
    g_txt = nc.dram_tensor("g_txt", [128, KC_TXT, 128], f16, kind="ExternalInput").ap()
    mout = nc.dram_tensor("mout", [128, 128], f16, kind="ExternalInput").ap()
    epair = nc.dram_tensor("epair", [128, F], bf16, kind="ExternalInput").ap()
    e2 = nc.dram_tensor("e2", [F, 128], bf16, kind="ExternalInput").ap()
    biases = nc.dram_tensor("biases", [128, 4], f32, kind="ExternalInput").ap()
    # output, block-major [D, img|txt, RB] per block, flat (graded RB)
    out = nc.dram_tensor(
        "out", [D * 2 * ROWS_CORE], f16, kind="ExternalOutput"
    ).ap()

    with tile.TileContext(nc) as tc, ExitStack() as ctx:
        singles = ctx.enter_context(tc.tile_pool(name="singles", bufs=1))
        xi_pool = ctx.enter_context(tc.tile_pool(name="xi", bufs=5))
        xt_pool = ctx.enter_context(tc.tile_pool(name="xt", bufs=5))
        s_pool = ctx.enter_context(tc.tile_pool(name="s", bufs=8))
        sq_pool = ctx.enter_context(tc.tile_pool(name="sq", bufs=4))
        mid_sb = ctx.enter_context(tc.tile_pool(name="midsb", bufs=2))
        r_pool = ctx.enter_context(tc.tile_pool(name="r", bufs=3))
        mask_pool = ctx.enter_context(tc.tile_pool(name="mask", bufs=4))
        osb_pool = ctx.enter_context(tc.tile_pool(name="osb", bufs=4))

        a_ps = ctx.enter_context(tc.tile_pool(name="aps", bufs=2, space="PSUM"))
        a2_ps = ctx.enter_context(tc.tile_pool(name="a2ps", bufs=2, space="PSUM"))
        g_ps = ctx.enter_context(tc.tile_pool(name="gps", bufs=2, space="PSUM"))
        o_ps = ctx.enter_context(tc.tile_pool(name="ops", bufs=1, space="PSUM"))

        # ---- g matrices first (needed by the first fwd); small consts are
        # posted after the first block loads to keep them off the start path
        g_img_sb = singles.tile([128, KC_IMG, 128], f16)
        nc.sync.dma_start(out=g_img_sb, in_=g_img)
        g_txt_sb = singles.tile([128, KC_TXT, 128], f16)
        nc.scalar.dma_start(out=g_txt_sb, in_=g_txt)
        mout_sb = singles.tile([128, 128], f16)
        epair_sb = singles.tile([128, F], bf16)
        e2_sb = singles.tile([F, 128], bf16)
        bias_sb = singles.tile([128, 4], f32)
        eps_sb = singles.tile([128, 1], f32)
        nc.vector.memset(eps_sb, EPS)

        def load_consts():
            nc.sync.dma_start(out=epair_sb, in_=epair)
            nc.sync.dma_start(out=bias_sb, in_=biases)
            nc.scalar.dma_start(out=e2_sb, in_=e2)
            nc.scalar.dma_start(out=mout_sb, in_=mout)

        warm_src = singles.tile([128, 128], f16, name="warm_src")
        nc.vector.memset(warm_src, 0.0)

        def pe_filler(n):
            """standalone weight loads: keep the PE active (HAM clock-gate
            releases with activity and re-throttles after ~3.4us idle)
            without touching PSUM or producing results"""
            for _ in range(n):
                nc.tensor.ldweights(weights=warm_src)

        X, SS, SQ, R, M = {}, {}, {}, {}, {}

        xi_ofs = [128 * KC_IMG * r0 for r0, _ in BLOCKS]
        xt_ofs = [128 * KC_TXT * r0 for r0, _ in BLOCKS]
        out_ofs = [128 * 2 * r0 for r0, _ in BLOCKS]

        def load(b):
            RB = BLOCKS[b][1]
            ei, et = (nc.sync, nc.scalar) if b % 2 == 0 else (nc.scalar, nc.sync)
            src_i = xi[xi_ofs[b] : xi_ofs[b] + 128 * KC_IMG * RB].rearrange(
                "(p f) -> p f", p=128
            )
            src_t = xt[xt_ofs[b] : xt_ofs[b] + 128 * KC_TXT * RB].rearrange(
                "(p f) -> p f", p=128
            )
            x_i = xi_pool.tile([128, KC_IMG * RB], f16, tag="xi")
            ei.dma_start(out=x_i, in_=src_i)
            x_t = xt_pool.tile([128, KC_TXT * RB], f16, tag="xt")
            et.dma_start(out=x_t, in_=src_t)
            X[b] = (x_i, x_t)

        def stage_fwd(b):
            """projection matmuls + bias/square (PE -> ACT/DVE)"""
            RB = BLOCKS[b][1]
            x_i, x_t = X.pop(b)
            A_i = a_ps.tile([128, RB], f32, tag="A", name="A_i")
            for kk in range(KC_IMG):
                nc.tensor.matmul(
                    A_i, g_img_sb[:, kk, :], x_i[:, kk * RB : (kk + 1) * RB],
                    start=(kk == 0), stop=(kk == KC_IMG - 1),
                )
            A_t = a_ps.tile([128, RB], f32, tag="A", name="A_t")
            for kk in range(KC_TXT):
                nc.tensor.matmul(
                    A_t, g_txt_sb[:, kk, :], x_t[:, kk * RB : (kk + 1) * RB],
                    start=(kk == 0), stop=(kk == KC_TXT - 1),
                )
            s_i = s_pool.tile([128, RB], f32, tag="s", name="s_i")
            nc.scalar.activation(
                out=s_i, in_=A_i, func=AF.Identity, bias=bias_sb[:, 0:1], scale=1.0
            )
            s_t = s_pool.tile([128, RB], f32, tag="s", name="s_t")
            nc.scalar.activation(
                out=s_t, in_=A_t, func=AF.Identity, bias=bias_sb[:, 1:2], scale=1.0
            )
            # squares on the PSUM-less Pool engine (SBUF->SBUF)
            sq_i = sq_pool.tile([128, RB], bf16, tag="sq", name="sq_i")
            nc.gpsimd.tensor_mul(sq_i, s_i, s_i)
            sq_t = sq_pool.tile([128, RB], bf16, tag="sq", name="sq_t")
            nc.gpsimd.tensor_mul(sq_t, s_t, s_t)
            SS[b] = (s_i, s_t)
            SQ[b] = (sq_i, sq_t)

        def stage_a2(j):
            """per-bin |A|^2 pair-sum + r = 1/(p+eps)  (PE -> DVE -> ACT)"""
            RB = BLOCKS[j][1]
            sq_i, sq_t = SQ.pop(j)
            a2_i = a2_ps.tile([F, RB], f32, tag="a2", name="a2_i")
            nc.tensor.matmul(a2_i, epair_sb, sq_i, start=True, stop=True)
            a2_t = a2_ps.tile([F, RB], f32, tag="a2", name="a2_t")
            nc.tensor.matmul(a2_t, epair_sb, sq_t, start=True, stop=True)
            a2t_sb = mid_sb.tile([F, RB], f32, tag="a2t", name="a2t_sb")
            nc.vector.tensor_copy(out=a2t_sb, in_=a2_t)
            p_sb = mid_sb.tile([F, RB], f32, tag="p", name="p_sb")
            nc.vector.tensor_mul(p_sb, a2_i, a2t_sb)
            u_sb = mid_sb.tile([F, RB], f32, tag="u", name="u_sb")
            nc.scalar.activation(
                out=u_sb, in_=p_sb, func=AF.Ln, bias=eps_sb[:F], scale=1.0
            )
            r_bf = r_pool.tile([F, RB], bf16, tag="rbf", name="r_bf")
            nc.scalar.activation(out=r_bf, in_=u_sb, func=AF.Exp, bias=0.0, scale=-1.0)
            R[j] = r_bf

        def stage_gexp(k):
            """bin->dim mask expand + masked spectra  (PE -> DVE).
            e2 is identical for both modalities, so one expand serves both."""
            RB = BLOCKS[k][1]
            r_bf = R.pop(k)
            s_i, s_t = SS.pop(k)
            gexp = g_ps.tile([128, RB], f32, tag="g", name="gexp")
            nc.tensor.matmul(gexp, e2_sb, r_bf, start=True, stop=True)
            mask_i = mask_pool.tile([128, RB], f16, tag="mask", name="mask_i")
            nc.vector.scalar_tensor_tensor(
                out=mask_i, in0=gexp, scalar=bias_sb[:, 2:3], in1=s_i,
                op0=mybir.AluOpType.add, op1=mybir.AluOpType.mult,
            )
            mask_t = mask_pool.tile([128, RB], f16, tag="mask", name="mask_t")
            nc.vector.scalar_tensor_tensor(
                out=mask_t, in0=gexp, scalar=bias_sb[:, 3:4], in1=s_t,
                op0=mybir.AluOpType.add, op1=mybir.AluOpType.mult,
            )
            M[k] = (mask_i, mask_t)

        def stage_out(l):
            """inverse transform + store  (PE -> ACT -> DMA).
            o is a [128, 2, RB_MAIN]-strided pair tile so both halves stay
            PSUM-bank-aligned; one copy + one store covers both modalities.
            Early stores ride the gpsimd SW queue (HW queues are busy
            loading); late ones use the then-idle HW queues."""
            RB = BLOCKS[l][1]
            mask_i, mask_t = M.pop(l)
            o_pair = o_ps.tile([128, 2, RB_MAIN], f32, tag="o", name="o_pair")
            nc.tensor.matmul(
                o_pair[:, 0, :RB], mout_sb, mask_i, start=True, stop=True
            )
            nc.tensor.matmul(
                o_pair[:, 1, :RB], mout_sb, mask_t, start=True, stop=True
            )
            osb = osb_pool.tile([128, 2, RB], f16, tag="osb", name="osb")
            nc.scalar.copy(out=osb[:, 0, :], in_=o_pair[:, 0, :RB])
            nc.vector.tensor_copy(out=osb[:, 1, :], in_=o_pair[:, 1, :RB])
            dst = out[out_ofs[l] : out_ofs[l] + 128 * 2 * RB].rearrange(
                "(p m r) -> p m r", p=128, m=2
            )
            eng = nc.gpsimd if l < 4 else (nc.sync if l % 2 == 0 else nc.scalar)
            eng.dma_start(out=dst, in_=osb)

        # tail stages issued BEFORE fwd: the PE drains ready work first and
        # every cross-engine dependency lands a full iteration ahead of use
        for i in range(NB + 2):
            if i < NB:
                load(i)
            if i == 0:
                pe_filler(55)   # ~6us warm-up during the load lead-in
            if i == 1:
                load_consts()
            if 0 <= i - 3 < NB:
                stage_a2(i - 3)
            if 0 <= i - 4 < NB:
                stage_gexp(i - 4)
            if 0 <= i - 5 < NB:
                stage_out(i - 5)
            if 0 <= i - 2 < NB:
                stage_fwd(i - 2)
            if 2 <= i <= 4:
                pe_filler(22)   # bridge early data-starved PE gaps
        # compressed epilogue: remaining stages tightly interleaved so the
        # drain costs ~2 sem-chains instead of 4 sparse iterations
        stage_a2(NB - 1)
        stage_gexp(NB - 2)
        stage_out(NB - 3)
        stage_gexp(NB - 1)
        stage_out(NB - 2)
        stage_out(NB - 1)

    _legalize_waits(nc)
    return nc


def _legalize_waits(nc):
    """This toolchain's walrus accepts at most ONE sync-wait command per
    engine instruction. Hoist excess waits onto same-engine EventSemaphore
    instructions inserted immediately before the offending instruction
    (engines execute their stream in order, so the carrier's wait gates
    the next instruction too)."""
    import bass_rust

    k = 0
    for f in nc.m.functions:
        for bb in f.blocks:
            new = []
            for ins in bb.instructions:
                si = getattr(ins, "sync_info", None)
                waits = list(si.on_wait) if si is not None and si.on_wait else []
                if len(waits) > 1:
                    for w in waits[:-1]:
                        nop = bass_rust.InstEventSemaphore(name=f"I-legalw-{k}")
                        k += 1
                        nop.engine = ins.engine
                        nop.sync_info = bass_rust.SyncInfo(on_wait=[w], on_update=[])
                        new.append(nop)
                    ins.sync_info = bass_rust.SyncInfo(
                        on_wait=[waits[-1]], on_update=list(si.on_update)
                    )
                new.append(ins)
            bb.instructions = new


LAST_RESULTS = None


def kernel(
    image_embeds,
    text_embeds,
    image_table,
    text_table,
    W_img,
    b_img,
    W_txt,
    b_txt,
    avrf_img,
    avrf_txt,
    avg_R,
    psi,
    lambda_weights,
):
    global LAST_RESULTS
    from concourse.bass_utils import run_bass_kernel_spmd

    f16 = np.float16
    import ml_dtypes

    bf = ml_dtypes.bfloat16
    image_table = np.asarray(image_table, np.float32)
    text_table = np.asarray(text_table, np.float32)
    W_img = np.asarray(W_img, np.float32)
    b_img = np.asarray(b_img, np.float32)
    W_txt = np.asarray(W_txt, np.float32)
    b_txt = np.asarray(b_txt, np.float32)
    avrf_img = np.asarray(avrf_img, np.float32)
    avrf_txt = np.asarray(avrf_txt, np.float32)
    avg_R = np.asarray(avg_R, np.float32)
    psi = np.asarray(psi, np.float32)
    lambda_weights = np.asarray(lambda_weights, np.float32)

    (GimgT, GtxtT, bias_img, bias_txt, Mout, lam0, lam1, epair, e2, bod) = _host_consts(
        W_img, b_img, W_txt, b_txt, avg_R, psi, lambda_weights
    )
    # per-partition constant term of the mask: cc[d] = c0[bod d] + c1
    cc_img = (lam0 * avrf_img[bod] + lam1).astype(np.float32)
    cc_txt = (lam0 * avrf_txt[bod] + lam1).astype(np.float32)
    biases = np.stack([bias_img, bias_txt, cc_img, cc_txt], axis=1).astype(
        np.float32
    )  # [128, 4]

    def gpack(g, kc):
        # [raw, 128] -> [k, chunk, m] fp16 (raw = chunk*128 + k)
        return np.ascontiguousarray(
            g.astype(f16).reshape(kc, 128, 128).transpose(1, 0, 2)
        )

    consts = dict(
        g_img=gpack(GimgT, KC_IMG),
        g_txt=gpack(GtxtT, KC_TXT),
        mout=np.ascontiguousarray(Mout.astype(f16)),
        epair=np.ascontiguousarray(epair, dtype=bf),
        e2=np.ascontiguousarray(e2, dtype=bf),
        biases=biases,
    )

    # fp16 transposed tables, packed block-major per core
    xiT = np.ascontiguousarray(image_table.astype(f16).T)   # [1024, 30000]
    xtT = np.ascontiguousarray(text_table.astype(f16).T)    # [768, 30000]

    def pack(xT, kc, lo):
        # core shard [raw, ROWS_CORE] -> flat block-major fp16
        flat = np.empty(kc * 128 * ROWS_CORE, f16)
        ofs = 0
        for r0, RB in BLOCKS:
            seg = xT[:, lo + r0 : lo + r0 + RB].reshape(kc, 128, RB)
            n = 128 * kc * RB
            flat[ofs : ofs + n] = seg.transpose(1, 0, 2).reshape(-1)
            ofs += n
        return flat

    if "nc" not in _CACHE:
        _CACHE["nc"] = _build_nc()
    nc = _CACHE["nc"]

    in_maps = []
    for c in range(N_CORES):
        lo = c * ROWS_CORE
        in_maps.append(
            dict(xi=pack(xiT, KC_IMG, lo), xt=pack(xtT, KC_TXT, lo), **consts)
        )

    res = run_bass_kernel_spmd(nc, in_maps, core_ids=list(range(N_CORES)))
    LAST_RESULTS = res

    def gather(half):
        cols = []
        for c in range(N_CORES):
            flat = res.results[c]["out"]        # flat block-major f16
            blks = []
            ofs = 0
            for r0, RB in BLOCKS:
                n = 128 * 2 * RB
                blks.append(flat[ofs : ofs + n].reshape(128, 2, RB)[:, half, :])
                ofs += n
            cols.append(np.concatenate(blks, axis=1).T)
        return np.concatenate(cols, axis=0).astype(np.float32)

    img = gather(0)
    txt = gather(1)
    return img, txt


# revision 36
# speedup vs baseline: 1.2154x; 1.2154x over previous
"""Trainium2 Bass kernel for the DAMPS spectral-fusion module.

Takes the full (unsharded) inputs, shards rows across 8 NeuronCores
(pure data parallel), and runs a fused matmul-only reformulation:

  proj + rFFT + phase-rotation  ->  one [raw, 128] matrix per modality
  (spectral packing: p0 = DC (unrotated), p1..63 = Re A_k, p64 = Nyquist
  (unrotated), p65..127 = Im A_k; cos(phi) for DC/Nyquist is folded into
  the output matrix because irfft ignores the imaginary part there)

  msc mask chain               ->  elementwise on [128, rows] tiles
                                   + tiny matmuls for the pair-sum
                                   (|A|^2) and the bin->dim mask expand
  masked irfft                 ->  one [128, 128] fp16 output matrix

Layout: the host pre-transposes each core's row-shard to [raw, rows]
in fp16 (half the HBM bytes of fp32; the 2e-2 tolerance leaves room)
and packs it block-major so each row-block is one fully contiguous
[128, kc*RB] region -> large DMA descriptors.

Schedule: software-pipelined across row blocks with a 3-deep stage
skew (fwd b | pair-sum b-1 | mask-expand b-2 | inverse b-3) so the
tensor engine never waits on the elementwise msc chain — every
cross-engine dependency gets a full iteration of slack.  This keeps
the PE HAM clock-gate warm and the DMA queues streaming.
"""

import math

import numpy as np

N_ITEMS = 30000
D = 128
F = 65
RAW_IMG = 1024
RAW_TXT = 768
N_CORES = 8
ROWS_CORE = N_ITEMS // N_CORES          # 3750
KC_IMG = RAW_IMG // 128                 # 8
KC_TXT = RAW_TXT // 128                 # 6
EPS = 1e-8

# row blocks per core: small first block starts the PE early; uniform after
RBS = [256, 512, 512, 512, 512, 512, 512, 422]
assert sum(RBS) == ROWS_CORE
NB = len(RBS)
RB_MAIN = 512
BLOCKS = []
_r0 = 0
for _rb in RBS:
    BLOCKS.append((_r0, _rb))
    _r0 += _rb

_CACHE = {}


def _bin_of_dim():
    """spectral dim (0..127) -> frequency bin (0..64)"""
    b = np.zeros(128, np.int64)
    b[0] = 0
    b[64] = 64
    b[1:64] = np.arange(1, 64)
    b[65:128] = np.arange(1, 64)
    return b


def _host_consts(W_img, b_img, W_txt, b_txt, avg_R, psi, lambda_weights):
    """Build the fused constant matrices (float64 internally)."""

    n = np.arange(D)
    k = np.arange(F)
    theta = 2.0 * np.pi * np.outer(k, n) / D          # [65, 128]
    phi = (avg_R.astype(np.float64) * 0.5 + psi.astype(np.float64))  # [65]
    s = 1.0 / math.sqrt(D)

    def dmat(sign):
        Dm = np.zeros((128, D))
        Dm[0, :] = s
        Dm[64, :] = s * np.cos(theta[64])
        a = theta[1:64] + sign * phi[1:64, None]
        Dm[1:64, :] = s * np.cos(a)
        Dm[65:128, :] = -s * np.sin(a)
        return Dm

    Dimg = dmat(+1.0)
    Dtxt = dmat(-1.0)

    GimgT = (W_img.astype(np.float64) @ Dimg.T).astype(np.float32)  # [1024,128]
    GtxtT = (W_txt.astype(np.float64) @ Dtxt.T).astype(np.float32)  # [768,128]
    bias_img = (Dimg @ b_img.astype(np.float64)).astype(np.float32)
    bias_txt = (Dtxt @ b_txt.astype(np.float64)).astype(np.float32)

    cphi = np.cos(phi)
    Mout = np.zeros((128, D))
    Mout[0, :] = s * cphi[0]
    Mout[64, :] = s * cphi[64] * np.cos(theta[64])
    Mout[1:64, :] = 2.0 * s * np.cos(theta[1:64])
    Mout[65:128, :] = -2.0 * s * np.sin(theta[1:64])
    Mout = Mout.astype(np.float32)

    lw = lambda_weights.astype(np.float64)
    e = np.exp(lw - lw.max())
    lam = e / e.sum()
    lam0, lam1 = float(lam[0]), float(lam[1])

    bod = _bin_of_dim()
    epair = np.zeros((128, F), np.float32)
    epair[np.arange(F), np.arange(F)] = 1.0            # cos^2 / DC / Nyq
    epair[64 + np.arange(1, 64), np.arange(1, 64)] = 1.0  # sin^2
    # e2 expand matrix maps r[bin] -> -c1*eps*r at each spectral dim; the
    # constant (c0[bin]+c1) term is applied later as a per-partition scalar:
    #   g_exp[d] = (c0[bod d]+c1) - c1*eps*r[bod d]   (== c0 + c1*msc)
    e2 = np.zeros((F, 128), np.float32)
    e2[bod, np.arange(128)] = -lam1 * EPS
    return GimgT, GtxtT, bias_img, bias_txt, Mout, lam0, lam1, epair, e2, bod


def _build_nc():
    """Build (once) the Bass/Tile program for one core's row shard."""
    from contextlib import ExitStack

    import concourse.bass as bass
    import concourse.tile as tile
    from concourse import mybir

    f32 = mybir.dt.float32
    f16 = mybir.dt.float16
    bf16 = mybir.dt.bfloat16
    AF = mybir.ActivationFunctionType

    nc = bass.Bass("TRN2", target_bir_lowering=False, debug=False)

    # block-major fp16 table shards, flat with per-block offsets (graded RB)
    xi = nc.dram_tensor(
        "xi", [128 * RAW_IMG // 128 * ROWS_CORE], f16, kind="ExternalInput"
    ).ap()
    xt = nc.dram_tensor(
        "xt", [128 * RAW_TXT // 128 * ROWS_CORE], f16, kind="ExternalInput"
    ).ap()
    # fused projection matrices, [k, chunk, m] fp16
    g_img = nc.dram_tensor("g_img", [128, KC_IMG, 128], f16, kind="ExternalInput").ap()
    g_txt = nc.dram_tensor("g_txt", [128, KC_TXT, 128], f16, kind="ExternalInput").ap()
    mout = nc.dram_tensor("mout", [128, 128], f16, kind="ExternalInput").ap()
    epair = nc.dram_tensor("epair", [128, F], bf16, kind="ExternalInput").ap()
    e2 = nc.dram_tensor("e2", [F, 128], bf16, kind="ExternalInput").ap()
    biases = nc.dram_tensor("biases", [128, 4], f32, kind="ExternalInput").ap()
    # output, block-major [D, img|txt, RB] per block, flat (graded RB)
    out = nc.dram_tensor(
        "out", [D * 2 * ROWS_CORE], f16, kind="ExternalOutput"
    ).ap()

    with tile.TileContext(nc) as tc, ExitStack() as ctx:
        singles = ctx.enter_context(tc.tile_pool(name="singles", bufs=1))
        xi_pool = ctx.enter_context(tc.tile_pool(name="xi", bufs=5))
        xt_pool = ctx.enter_context(tc.tile_pool(name="xt", bufs=5))
        s_pool = ctx.enter_context(tc.tile_pool(name="s", bufs=8))
        sq_pool = ctx.enter_context(tc.tile_pool(name="sq", bufs=4))
        mid_sb = ctx.enter_context(tc.tile_pool(name="midsb", bufs=2))
        r_pool = ctx.enter_context(tc.tile_pool(name="r", bufs=3))
        mask_pool = ctx.enter_context(tc.tile_pool(name="mask", bufs=4))
        osb_pool = ctx.enter_context(tc.tile_pool(name="osb", bufs=4))

        a_ps = ctx.enter_context(tc.tile_pool(name="aps", bufs=2, space="PSUM"))
        a2_ps = ctx.enter_context(tc.tile_pool(name="a2ps", bufs=2, space="PSUM"))
        g_ps = ctx.enter_context(tc.tile_pool(name="gps", bufs=2, space="PSUM"))
        o_ps = ctx.enter_context(tc.tile_pool(name="ops", bufs=1, space="PSUM"))

        # ---- g matrices first (needed by the first fwd); small consts are
        # posted after the first block loads to keep them off the start path
        g_img_sb = singles.tile([128, KC_IMG, 128], f16)
        nc.sync.dma_start(out=g_img_sb, in_=g_img)
        g_txt_sb = singles.tile([128, KC_TXT, 128], f16)
        nc.scalar.dma_start(out=g_txt_sb, in_=g_txt)
        mout_sb = singles.tile([128, 128], f16)
        epair_sb = singles.tile([128, F], bf16)
        e2_sb = singles.tile([F, 128], bf16)
        bias_sb = singles.tile([128, 4], f32)
        eps_sb = singles.tile([128, 1], f32)
        nc.vector.memset(eps_sb, EPS)

        def load_consts():
            nc.sync.dma_start(out=epair_sb, in_=epair)
            nc.sync.dma_start(out=bias_sb, in_=biases)
            nc.scalar.dma_start(out=e2_sb, in_=e2)
            nc.scalar.dma_start(out=mout_sb, in_=mout)

        warm_src = singles.tile([128, 128], f16, name="warm_src")
        nc.vector.memset(warm_src, 0.0)

        def pe_filler(n):
            """standalone weight loads: keep the PE active (HAM clock-gate
            releases with activity and re-throttles after ~3.4us idle)
            without touching PSUM or producing results"""
            for _ in range(n):
                nc.tensor.ldweights(weights=warm_src)

        X, SS, SQ, R, M = {}, {}, {}, {}, {}

        xi_ofs = [128 * KC_IMG * r0 for r0, _ in BLOCKS]
        xt_ofs = [128 * KC_TXT * r0 for r0, _ in BLOCKS]
        out_ofs = [128 * 2 * r0 for r0, _ in BLOCKS]

        def load(b):
            RB = BLOCKS[b][1]
            ei, et = (nc.sync, nc.scalar) if b % 2 == 0 else (nc.scalar, nc.sync)
            src_i = xi[xi_ofs[b] : xi_ofs[b] + 128 * KC_IMG * RB].rearrange(
                "(p f) -> p f", p=128
            )
            src_t = xt[xt_ofs[b] : xt_ofs[b] + 128 * KC_TXT * RB].rearrange(
                "(p f) -> p f", p=128
            )
            x_i = xi_pool.tile([128, KC_IMG * RB], f16, tag="xi")
            ei.dma_start(out=x_i, in_=src_i)
            x_t = xt_pool.tile([128, KC_TXT * RB], f16, tag="xt")
            et.dma_start(out=x_t, in_=src_t)
            X[b] = (x_i, x_t)

        def stage_fwd(b):
            """projection matmuls + bias/square (PE -> ACT/DVE)"""
            RB = BLOCKS[b][1]
            x_i, x_t = X.pop(b)
            A_i = a_ps.tile([128, RB], f32, tag="A", name="A_i")
            for kk in range(KC_IMG):
                nc.tensor.matmul(
                    A_i, g_img_sb[:, kk, :], x_i[:, kk * RB : (kk + 1) * RB],
                    start=(kk == 0), stop=(kk == KC_IMG - 1),
                )
            A_t = a_ps.tile([128, RB], f32, tag="A", name="A_t")
            for kk in range(KC_TXT):
                nc.tensor.matmul(
                    A_t, g_txt_sb[:, kk, :], x_t[:, kk * RB : (kk + 1) * RB],
                    start=(kk == 0), stop=(kk == KC_TXT - 1),
                )
            s_i = s_pool.tile([128, RB], f32, tag="s", name="s_i")
            nc.scalar.activation(
                out=s_i, in_=A_i, func=AF.Identity, bias=bias_sb[:, 0:1], scale=1.0
            )
            s_t = s_pool.tile([128, RB], f32, tag="s", name="s_t")
            nc.scalar.activation(
                out=s_t, in_=A_t, func=AF.Identity, bias=bias_sb[:, 1:2], scale=1.0
            )
            # squares on the PSUM-less Pool engine (SBUF->SBUF)
            sq_i = sq_pool.tile([128, RB], bf16, tag="sq", name="sq_i")
            nc.gpsimd.tensor_mul(sq_i, s_i, s_i)
            sq_t = sq_pool.tile([128, RB], bf16, tag="sq", name="sq_t")
            nc.gpsimd.tensor_mul(sq_t, s_t, s_t)
            SS[b] = (s_i, s_t)
            SQ[b] = (sq_i, sq_t)

        def stage_a2(j):
            """per-bin |A|^2 pair-sum + r = 1/(p+eps)  (PE -> DVE -> ACT)"""
            RB = BLOCKS[j][1]
            sq_i, sq_t = SQ.pop(j)
            a2_i = a2_ps.tile([F, RB], f32, tag="a2", name="a2_i")
            nc.tensor.matmul(a2_i, epair_sb, sq_i, start=True, stop=True)
            a2_t = a2_ps.tile([F, RB], f32, tag="a2", name="a2_t")
            nc.tensor.matmul(a2_t, epair_sb, sq_t, start=True, stop=True)
            a2t_sb = mid_sb.tile([F, RB], f32, tag="a2t", name="a2t_sb")
            nc.vector.tensor_copy(out=a2t_sb, in_=a2_t)
            p_sb = mid_sb.tile([F, RB], f32, tag="p", name="p_sb")
            nc.vector.tensor_mul(p_sb, a2_i, a2t_sb)
            u_sb = mid_sb.tile([F, RB], f32, tag="u", name="u_sb")
            nc.scalar.activation(
                out=u_sb, in_=p_sb, func=AF.Ln, bias=eps_sb[:F], scale=1.0
            )
            r_bf = r_pool.tile([F, RB], bf16, tag="rbf", name="r_bf")
            nc.scalar.activation(out=r_bf, in_=u_sb, func=AF.Exp, bias=0.0, scale=-1.0)
            R[j] = r_bf

        def stage_gexp(k):
            """bin->dim mask expand + masked spectra  (PE -> DVE).
            e2 is identical for both modalities, so one expand serves both."""
            RB = BLOCKS[k][1]
            r_bf = R.pop(k)
            s_i, s_t = SS.pop(k)
            gexp = g_ps.tile([128, RB], f32, tag="g", name="gexp")
            nc.tensor.matmul(gexp, e2_sb, r_bf, start=True, stop=True)
            mask_i = mask_pool.tile([128, RB], f16, tag="mask", name="mask_i")
            nc.vector.scalar_tensor_tensor(
                out=mask_i, in0=gexp, scalar=bias_sb[:, 2:3], in1=s_i,
                op0=mybir.AluOpType.add, op1=mybir.AluOpType.mult,
            )
            mask_t = mask_pool.tile([128, RB], f16, tag="mask", name="mask_t")
            nc.vector.scalar_tensor_tensor(
                out=mask_t, in0=gexp, scalar=bias_sb[:, 3:4], in1=s_t,
                op0=mybir.AluOpType.add, op1=mybir.AluOpType.mult,
            )
            M[k] = (mask_i, mask_t)

        def stage_out(l):
            """inverse transform + store  (PE -> ACT -> DMA).
            o is a [128, 2, RB_MAIN]-strided pair tile so both halves stay
            PSUM-bank-aligned; one copy + one store covers both modalities.
            Early stores ride the gpsimd SW queue (HW queues are busy
            loading); late ones use the then-idle HW queues."""
            RB = BLOCKS[l][1]
            mask_i, mask_t = M.pop(l)
            o_pair = o_ps.tile([128, 2, RB_MAIN], f32, tag="o", name="o_pair")
            nc.tensor.matmul(
                o_pair[:, 0, :RB], mout_sb, mask_i, start=True, stop=True
            )
            nc.tensor.matmul(
                o_pair[:, 1, :RB], mout_sb, mask_t, start=True, stop=True
            )
            osb = osb_pool.tile([128, 2, RB], f16, tag="osb", name="osb")
            nc.scalar.copy(out=osb[:, 0, :], in_=o_pair[:, 0, :RB])
            nc.vector.tensor_copy(out=osb[:, 1, :], in_=o_pair[:, 1, :RB])
            dst = out[out_ofs[l] : out_ofs[l] + 128 * 2 * RB].rearrange(
                "(p m r) -> p m r", p=128, m=2
            )
            eng = nc.gpsimd if l < 4 else (nc.sync if l % 2 == 0 else nc.scalar)
            eng.dma_start(out=dst, in_=osb)

        # tail stages issued BEFORE fwd: the PE drains ready work first and
        # every cross-engine dependency lands a full iteration ahead of use
        for i in range(NB + 2):
            if i < NB:
                load(i)
            if i == 1:
                load_consts()
            if 0 <= i - 3 < NB:
                stage_a2(i - 3)
            if 0 <= i - 4 < NB:
                stage_gexp(i - 4)
            if 0 <= i - 5 < NB:
                stage_out(i - 5)
            if 0 <= i - 2 < NB:
                stage_fwd(i - 2)
        # compressed epilogue: remaining stages tightly interleaved so the
        # drain costs ~2 sem-chains instead of 4 sparse iterations
        stage_a2(NB - 1)
        stage_gexp(NB - 2)
        stage_out(NB - 3)
        stage_gexp(NB - 1)
        stage_out(NB - 2)
        stage_out(NB - 1)

    _legalize_waits(nc)
    return nc


def _legalize_waits(nc):
    """This toolchain's walrus accepts at most ONE sync-wait command per
    engine instruction. Hoist excess waits onto same-engine EventSemaphore
    instructions inserted immediately before the offending instruction
    (engines execute their stream in order, so the carrier's wait gates
    the next instruction too)."""
    import bass_rust

    k = 0
    for f in nc.m.functions:
        for bb in f.blocks:
            new = []
            for ins in bb.instructions:
                si = getattr(ins, "sync_info", None)
                waits = list(si.on_wait) if si is not None and si.on_wait else []
                if len(waits) > 1:
                    for w in waits[:-1]:
                        nop = bass_rust.InstEventSemaphore(name=f"I-legalw-{k}")
                        k += 1
                        nop.engine = ins.engine
                        nop.sync_info = bass_rust.SyncInfo(on_wait=[w], on_update=[])
                        new.append(nop)
                    ins.sync_info = bass_rust.SyncInfo(
                        on_wait=[waits[-1]], on_update=list(si.on_update)
                    )
                new.append(ins)
            bb.instructions = new


LAST_RESULTS = None


def kernel(
    image_embeds,
    text_embeds,
    image_table,
    text_table,
    W_img,
    b_img,
    W_txt,
    b_txt,
    avrf_img,
    avrf_txt,
    avg_R,
    psi,
    lambda_weights,
):
    global LAST_RESULTS
    from concourse.bass_utils import run_bass_kernel_spmd

    f16 = np.float16
    import ml_dtypes

    bf = ml_dtypes.bfloat16
    image_table = np.asarray(image_table, np.float32)
    text_table = np.asarray(text_table, np.float32)
    W_img = np.asarray(W_img, np.float32)
    b_img = np.asarray(b_img, np.float32)
    W_txt = np.asarray(W_txt, np.float32)
    b_txt = np.asarray(b_txt, np.float32)
    avrf_img = np.asarray(avrf_img, np.float32)
    avrf_txt = np.asarray(avrf_txt, np.float32)
    avg_R = np.asarray(avg_R, np.float32)
    psi = np.asarray(psi, np.float32)
    lambda_weights = np.asarray(lambda_weights, np.float32)

    (GimgT, GtxtT, bias_img, bias_txt, Mout, lam0, lam1, epair, e2, bod) = _host_consts(
        W_img, b_img, W_txt, b_txt, avg_R, psi, lambda_weights
    )
    # per-partition constant term of the mask: cc[d] = c0[bod d] + c1
    cc_img = (lam0 * avrf_img[bod] + lam1).astype(np.float32)
    cc_txt = (lam0 * avrf_txt[bod] + lam1).astype(np.float32)
    biases = np.stack([bias_img, bias_txt, cc_img, cc_txt], axis=1).astype(
        np.float32
    )  # [128, 4]

    def gpack(g, kc):
        # [raw, 128] -> [k, chunk, m] fp16 (raw = chunk*128 + k)
        return np.ascontiguousarray(
            g.astype(f16).reshape(kc, 128, 128).transpose(1, 0, 2)
        )

    consts = dict(
        g_img=gpack(GimgT, KC_IMG),
        g_txt=gpack(GtxtT, KC_TXT),
        mout=np.ascontiguousarray(Mout.astype(f16)),
        epair=np.ascontiguousarray(epair, dtype=bf),
        e2=np.ascontiguousarray(e2, dtype=bf),
        biases=biases,
    )

    # fp16 transposed tables, packed block-major per core
    xiT = np.ascontiguousarray(image_table.astype(f16).T)   # [1024, 30000]
    xtT = np.ascontiguousarray(text_table.astype(f16).T)    # [768, 30000]

    def pack(xT, kc, lo):
        # core shard [raw, ROWS_CORE] -> flat block-major fp16
        flat = np.empty(kc * 128 * ROWS_CORE, f16)
        ofs = 0
        for r0, RB in BLOCKS:
            seg = xT[:, lo + r0 : lo + r0 + RB].reshape(kc, 128, RB)
            n = 128 * kc * RB
            flat[ofs : ofs + n] = seg.transpose(1, 0, 2).reshape(-1)
            ofs += n
        return flat

    if "nc" not in _CACHE:
        _CACHE["nc"] = _build_nc()
    nc = _CACHE["nc"]

    in_maps = []
    for c in range(N_CORES):
        lo = c * ROWS_CORE
        in_maps.append(
            dict(xi=pack(xiT, KC_IMG, lo), xt=pack(xtT, KC_TXT, lo), **consts)
        )

    res = run_bass_kernel_spmd(nc, in_maps, core_ids=list(range(N_CORES)))
    LAST_RESULTS = res

    def gather(half):
        cols = []
        for c in range(N_CORES):
            flat = res.results[c]["out"]        # flat block-major f16
            blks = []
            ofs = 0
            for r0, RB in BLOCKS:
                n = 128 * 2 * RB
                blks.append(flat[ofs : ofs + n].reshape(128, 2, RB)[:, half, :])
                ofs += n
            cols.append(np.concatenate(blks, axis=1).T)
        return np.concatenate(cols, axis=0).astype(np.float32)

    img = gather(0)
    txt = gather(1)
    return img, txt
